# revision 1
# baseline (speedup 1.0000x reference)
"""Trainium2 Bass kernel for nn_CausalSelfAttention_16810501996824.

Head-sharded (tensor-parallel) causal self-attention over 8 NeuronCores:
each core owns 2 of the 16 heads end-to-end (QKV projection, RMS norm,
rotary, causal attention with sigmoid gate and lambda-blended V), then an
AllGather of the per-head context vectors and an output-column-sharded
c_proj. Host only reshapes/concats shards.

Self-contained: hardcodes all shapes; builds + compiles the Bass module on
first call and caches the jitted SPMD executable.
"""
import json

import numpy as np

# ---------------------------------------------------------------------------
# Problem constants
# ---------------------------------------------------------------------------
DIM = 1024
N_HEAD = 16
T = 2048
HD = 64                 # head dim
GATE_IN = 12
ROPE_BASE = 10000.0
ATTN_SCALE = 0.1
EPS = 1e-6
N_CORES = 8
HPC = N_HEAD // N_CORES  # heads per core = 2
C = HPC * HD             # channels per core = 128
NT512 = T // 512         # 4 t-windows
NS128 = T // 128         # 16 s-blocks

# ---------------------------------------------------------------------------
# Workaround: the staged walrus build allows at most 1 sem wait per
# instruction (2 for EventSemaphore); stock Tile piles multiple waits onto
# one instruction. Split extras onto single-wait NoOps at serialization.
# ---------------------------------------------------------------------------
_WAIT_CAP = {"EventSemaphore": 2}


def _split_multi_waits(bir: dict) -> dict:
    for fn in bir.get("functions", []):
        for blk in fn.get("blocks", []):
            out = []
            changed = False
            for inst in blk.get("instructions", []):
                si = inst.get("sync_info") or {}
                waits = si.get("on_wait") or []
                cap = _WAIT_CAP.get(inst.get("opcode"), 1)
                if len(waits) > cap:
                    changed = True
                    for j, w in enumerate(waits[cap:]):
                        out.append({
                            "debug": inst.get("debug", 0),
                            "engine": inst["engine"],
                            "ins": [], "outs": [],
                            "name": f"{inst['name']}-wsplit{j}",
                            "opcode": "NoOp",
                            "sync_info": {"on_update": [], "on_wait": [w]},
                            "text_hint": "wait_split",
                        })
                    si = dict(si)
                    si["on_wait"] = waits[:cap]
                    inst = dict(inst)
                    inst["sync_info"] = si
                out.append(inst)
            if changed:
                blk["instructions"] = out
    return bir


def _install_patches():
    import concourse.bass as bass
    if getattr(bass.Bass, "_wait_split_patched", False):
        return
    orig = bass.Bass.to_json_bytes

    def patched(self, *a, **k):
        return json.dumps(_split_multi_waits(json.loads(orig(self, *a, **k)))).encode()

    bass.Bass.to_json_bytes = patched
    bass.Bass._wait_split_patched = True


# ---------------------------------------------------------------------------
# Bass module
# ---------------------------------------------------------------------------

def _build_module(repeat=1, phases=4):
    import concourse.bass as bass
    import concourse.mybir as mybir
    import concourse.tile as tile

    F32 = mybir.dt.float32
    F32R = mybir.dt.float32r
    AF = mybir.ActivationFunctionType

    nc = bass.Bass()

    xT = nc.declare_dram_parameter("xT", [DIM, T], F32R, isOutput=False)
    wqkvT = nc.declare_dram_parameter("wqkvT", [DIM, 3 * C], F32R, isOutput=False)
    wgT = nc.declare_dram_parameter("wgT", [GATE_IN, HPC], F32R, isOutput=False)
    wprojT = nc.declare_dram_parameter("wprojT", [DIM, C], F32R, isOutput=False)
    v1lam = nc.declare_dram_parameter("v1lam", [T, C], F32R, isOutput=False)
    lam1 = nc.declare_dram_parameter("lam1", [128, 1], F32, isOutput=False)
    cosd = nc.declare_dram_parameter("cosd", [C, T], F32, isOutput=False)
    sind = nc.declare_dram_parameter("sind", [C, T], F32, isOutput=False)
    swapm = nc.declare_dram_parameter("swapm", [128, 128], F32R, isOutput=False)
    identm = nc.declare_dram_parameter("identm", [128, 128], F32R, isOutput=False)
    outT = nc.declare_dram_parameter("outT", [C, T], F32, isOutput=True)

    HT = T // 2
    y_loc = [nc.dram_tensor(f"y_loc{i}", [C, HT], F32R) for i in range(2)]
    y_full = [nc.dram_tensor(f"y_full{i}", [N_CORES * C, HT], F32R, addr_space="Shared")
              for i in range(2)]

    with nc.allow_low_precision(reason="f32r matmul pipeline"), \
            tile.TileContext(nc) as tc:
      for _rep in range(repeat):
        with tc.tile_pool(name=f"persist{_rep}", bufs=1) as persist, \
             tc.tile_pool(name=f"vaug{_rep}", bufs=1) as vaug_pool:
            # ---- persistent tiles ----
            qt = persist.tile([128, T], F32R)      # qT, both heads stacked
            kt = persist.tile([128, T], F32R)
            vt = persist.tile([128, T], F32R)      # vT (pre-blend)
            gtmp = persist.tile([HPC, T], F32R)
            ident = persist.tile([128, 128], F32R)
            swp = persist.tile([128, 128], F32R)
            ones_col = persist.tile([65, 64], F32R)
            mlo = persist.tile([1, 128], F32R)
            mhi = persist.tile([1, 128], F32R)
            blo = persist.tile([128, 1], F32R)
            bhi = persist.tile([128, 1], F32R)
            lam1_sb = persist.tile([128, 1], F32)
            eps_sb = persist.tile([128, 1], F32)
            v_aug = [[vaug_pool.tile([128, HD + 1], F32R, name=f"va{h}_{si}", tag=f"va{h}_{si}")
                      for si in range(NS128)] for h in range(HPC)]

            nc.sync.dma_start(out=swp, in_=swapm[:])
            nc.sync.dma_start(out=lam1_sb, in_=lam1[:])
            nc.vector.memset(eps_sb, EPS)
            nc.sync.dma_start(out=ident, in_=identm[:])
            nc.vector.memset(ones_col.bitcast(F32), 1.0)
            nc.vector.memset(mlo.bitcast(F32), 0.0)
            nc.vector.memset(mlo.bitcast(F32)[:, 0:64], 1.0)
            nc.vector.memset(mhi.bitcast(F32), 0.0)
            nc.vector.memset(mhi.bitcast(F32)[:, 64:128], 1.0)
            nc.vector.memset(blo.bitcast(F32), 0.0)
            nc.vector.memset(blo.bitcast(F32)[0:64, :], 1.0 / HD)
            nc.vector.memset(bhi.bitcast(F32), 0.0)
            nc.vector.memset(bhi.bitcast(F32)[64:128, :], 1.0 / HD)

            # =============================================================
            # Phase 1: QKV projections + RMS norm + rotary + gate
            # =============================================================
            with tc.tile_pool(name="p1sbuf", bufs=1) as p1, \
                 tc.tile_pool(name="p1temp", bufs=2) as p1t, \
                 tc.tile_pool(name="p1acc", bufs=1, space="PSUM") as p1acc, \
                 tc.tile_pool(name="p1aux", bufs=2, space="PSUM") as p1aux, \
                 tc.tile_pool(name="p1small", bufs=3, space="PSUM") as p1s:
                xts = [p1.tile([128, T], F32R, name=f"xt{d}", tag=f"xt{d}") for d in range(8)]
                wts = [p1.tile([128, 3 * C], F32R, name=f"wt{d}", tag=f"wt{d}") for d in range(8)]
                wg_sb = p1.tile([GATE_IN, HPC], F32R)
                for d in range(8):
                    nc.gpsimd.dma_start(out=xts[d], in_=xT[128 * d:128 * (d + 1), :])
                    nc.gpsimd.dma_start(out=wts[d], in_=wqkvT[128 * d:128 * (d + 1), :])
                nc.sync.dma_start(out=wg_sb, in_=wgT[:])

                for tj in range(NT512):
                    ts = slice(512 * tj, 512 * (tj + 1))
                    cos_sb = p1t.tile([C, 512], F32, tag="cos")
                    sin_sb = p1t.tile([C, 512], F32, tag="sin")
                    nc.gpsimd.dma_start(out=cos_sb, in_=cosd[:, ts])
                    nc.gpsimd.dma_start(out=sin_sb, in_=sind[:, ts])

                    q_ps = p1acc.tile([128, 512], F32, tag="q_ps")
                    k_ps = p1acc.tile([128, 512], F32, tag="k_ps")
                    v_ps = p1acc.tile([128, 512], F32, tag="v_ps")
                    for d in range(8):
                        nc.tensor.matmul(q_ps, wts[d][:, 0:128], xts[d][:, ts],
                                         start=(d == 0), stop=(d == 7))
                    for d in range(8):
                        nc.tensor.matmul(k_ps, wts[d][:, 128:256], xts[d][:, ts],
                                         start=(d == 0), stop=(d == 7))
                    for d in range(8):
                        nc.tensor.matmul(v_ps, wts[d][:, 256:384], xts[d][:, ts],
                                         start=(d == 0), stop=(d == 7))
                    nc.scalar.copy(vt[:, ts], v_ps)

                    # gate: one M=2 matmul (K=12), sigmoid into gtmp rows 0,1
                    g_ps = p1s.tile([HPC, 512], F32, tag="sm")
                    nc.tensor.matmul(g_ps, wg_sb, xts[0][0:GATE_IN, ts],
                                     start=True, stop=True)
                    nc.scalar.activation(gtmp[:, ts], g_ps, AF.Sigmoid)

                    for name, r_ps, dst in (("q", q_ps, qt), ("k", k_ps, kt)):
                        raw = p1t.tile([128, 512], F32R, tag=f"{name}raw")
                        nc.scalar.copy(raw, r_ps)
                        sq = p1t.tile([128, 512], F32R, tag=f"{name}sq")
                        nc.scalar.activation(sq, r_ps, AF.Square)
                        ms0 = p1s.tile([1, 512], F32, tag="sm")
                        ms1 = p1s.tile([1, 512], F32, tag="sm")
                        nc.tensor.matmul(ms0, blo, sq, start=True, stop=True)
                        nc.tensor.matmul(ms1, bhi, sq, start=True, stop=True)
                        rt0 = p1t.tile([1, 512], F32R, tag=f"{name}rt0")
                        rt1 = p1t.tile([1, 512], F32R, tag=f"{name}rt1")
                        nc.scalar.activation(rt0, ms0, AF.Sqrt, bias=eps_sb[0:1, :])
                        nc.scalar.activation(rt1, ms1, AF.Sqrt, bias=eps_sb[0:1, :])
                        nc.vector.reciprocal(rt0, rt0)
                        nc.vector.reciprocal(rt1, rt1)
                        bc_ps = p1aux.tile([128, 512], F32, tag="aux")
                        nc.tensor.matmul(bc_ps, mlo, rt0, start=True, stop=False)
                        nc.tensor.matmul(bc_ps, mhi, rt1, start=False, stop=True)
                        sw_ps = p1aux.tile([128, 512], F32, tag="aux")
                        nc.tensor.matmul(sw_ps, swp, raw, start=True, stop=True)
                        t1 = p1t.tile([128, 512], F32, tag=f"{name}t1")
                        nc.vector.tensor_mul(t1, raw.bitcast(F32), cos_sb)
                        t2 = p1t.tile([128, 512], F32, tag=f"{name}t2")
                        nc.vector.tensor_mul(t2, sw_ps, sin_sb)
                        nc.vector.tensor_add(t1, t1, t2)
                        nc.vector.tensor_mul(dst[:, ts], t1.bitcast(F32R),
                                             bc_ps.bitcast(F32R))

                # vT -> v_nat blocks (+ lambda blend + ones col)
                for si in range(NS128):
                    ss = slice(128 * si, 128 * (si + 1))
                    tr_ps = p1aux.tile([128, 128], F32R, tag="aux")
                    nc.tensor.transpose(tr_ps, vt[:, ss], ident)
                    vl = p1t.tile([128, C], F32R, tag="vl")
                    nc.gpsimd.dma_start(out=vl, in_=v1lam[ss, :])
                    for h in range(HPC):
                        va = v_aug[h][si]
                        nc.scalar.activation(va[:, 0:HD], tr_ps[:, HD * h:HD * (h + 1)],
                                             AF.Copy, scale=lam1_sb)
                        nc.vector.tensor_add(va[:, 0:HD], va[:, 0:HD],
                                             vl[:, HD * h:HD * (h + 1)])
                        nc.vector.memset(va.bitcast(F32)[:, HD:HD + 1], 1.0)

            # =============================================================
            # Phase 2: causal attention per head
            # =============================================================
            if phases < 2:
                nc.sync.dma_start(out=outT[:].bitcast(F32R), in_=qt)
                continue
            with tc.tile_pool(name="p2t", bufs=3) as p2t, \
                 tc.tile_pool(name="p2small", bufs=2) as p2s, \
                 tc.tile_pool(name="masks", bufs=1) as mask_pool, \
                 tc.tile_pool(name="sps", bufs=4, space="PSUM") as sps_pool, \
                 tc.tile_pool(name="yps", bufs=2, space="PSUM") as yps_pool, \
                 tc.tile_pool(name="bcps", bufs=2, space="PSUM") as bcps_pool:
                # causal 0/1 masks for the 4 diagonal-band offsets, built once
                dmask = [mask_pool.tile([128, 512], F32R, name=f"dm{k}", tag=f"dm{k}")
                         for k in range(4)]
                for k in range(4):
                    nc.vector.memset(dmask[k].bitcast(F32), 1.0)
                    nc.gpsimd.affine_select(
                        out=dmask[k], in_=dmask[k],
                        compare_op=mybir.AluOpType.is_ge,
                        fill=0.0, base=-(128 * k),
                        channel_multiplier=-1, pattern=[[1, 512]])
                for tj in range(NT512):
                    ts = slice(512 * tj, 512 * (tj + 1))
                    hts = slice(512 * (tj % 2), 512 * (tj % 2) + 512)
                    for h in range(HPC):
                        hs = slice(HD * h, HD * (h + 1))
                        nsb = 4 * tj + 4
                        g64 = p2s.tile([65, 512], F32R, tag="g64")
                        nc.sync.dma_start(out=g64[64:65, :], in_=gtmp[h:h + 1, ts])
                        y_ps = yps_pool.tile([65, 512], F32, tag="y")
                        pending = None
                        for si in range(nsb):
                            ss = slice(128 * si, 128 * (si + 1))
                            s_ps = sps_pool.tile([128, 512], F32, tag="s")
                            nc.tensor.matmul(s_ps, kt[hs, ss], qt[hs, ts],
                                             start=True, stop=True)
                            p_sb = p2t.tile([128, 512], F32R, tag="p", bufs=4)
                            nc.scalar.activation(p_sb, s_ps, AF.Exp, scale=ATTN_SCALE)
                            ko = si - 4 * tj
                            if ko >= 0:
                                nc.vector.tensor_mul(p_sb, p_sb, dmask[ko])
                            if pending is not None:
                                psi, pp = pending
                                nc.tensor.matmul(y_ps, v_aug[h][psi], pp,
                                                 start=(psi == 0), stop=False)
                            pending = (si, p_sb)
                        psi, pp = pending
                        nc.tensor.matmul(y_ps, v_aug[h][psi], pp,
                                         start=(psi == 0), stop=True)
                        u = p2s.tile([65, 512], F32R, tag="u")
                        nc.vector.reciprocal(u[64:65, :], y_ps[64:65, :])
                        cs_row = p2s.tile([65, 512], F32R, tag="cs")
                        nc.vector.tensor_mul(cs_row[64:65, :], u[64:65, :],
                                             g64[64:65, :])
                        bc_ps = bcps_pool.tile([64, 512], F32, tag="bc")
                        nc.tensor.matmul(bc_ps, ones_col[64:65, :], cs_row[64:65, :],
                                         start=True, stop=True)
                        cs_sb = p2s.tile([64, 512], F32, tag="csb")
                        nc.scalar.copy(cs_sb, bc_ps)
                        yft = p2t.tile([64, 512], F32R, tag="yft")
                        nc.vector.tensor_mul(yft, y_ps[0:64, :], cs_sb)
                        nc.scalar.dma_start(out=y_loc[tj // 2][64 * h:64 * (h + 1), hts],
                                            in_=yft)
                    if tj == 1 and phases >= 3:
                        nc.gpsimd.collective_compute(
                            "AllGather", mybir.AluOpType.bypass,
                            ins=[y_loc[0][:]], outs=[y_full[0][:]],
                            replica_groups=[list(range(N_CORES))],
                        )

            # =============================================================
            # Phase 3: AllGather (second half; first half fired inside P2)
            # =============================================================
            if phases < 3:
                nc.sync.dma_start(out=outT[:, 0:HT].bitcast(F32R), in_=y_loc[0][:])
                continue
            # =============================================================
            # Phase 4: output projection (e-slice of out^T), interleaved
            # with the second-half AllGather so its blocking wait on the
            # gpsimd queue hides behind the half-0 loads + matmuls.
            # =============================================================
            if phases < 4:
                nc.gpsimd.collective_compute(
                    "AllGather", mybir.AluOpType.bypass,
                    ins=[y_loc[1][:]], outs=[y_full[1][:]],
                    replica_groups=[list(range(N_CORES))],
                )
                nc.sync.dma_start(out=outT[:, 0:HT].bitcast(F32R),
                                  in_=y_full[0][0:C, :])
                continue
            with tc.tile_pool(name="p4", bufs=3) as p4, \
                 tc.tile_pool(name="p4o", bufs=1) as p4o, \
                 tc.tile_pool(name="p4w", bufs=1) as p4w, \
                 tc.tile_pool(name="ops", bufs=1, space="PSUM") as ops_pool:
                wp = [p4w.tile([128, C], F32R, name=f"wp{cc}", tag=f"wp{cc}") for cc in range(8)]
                for cc in range(8):
                    nc.sync.dma_start(out=wp[cc], in_=wprojT[128 * cc:128 * (cc + 1), :])
                o_ps = [ops_pool.tile([128, 512], F32, name=f"o{tj}", tag=f"o{tj}")
                        for tj in range(NT512)]
                yfc = p4.tile([128, 8, T], F32R, tag="yfc", bufs=1)
                yfv = [y_full[i].rearrange("(cc p) t -> p cc t", p=128) for i in range(2)]
                for cc in range(8):
                    nc.gpsimd.dma_start(out=yfc[:, cc, 0:HT], in_=yfv[0][:, cc, :])
                for cc in range(8):
                    for tj in (0, 1):
                        nc.tensor.matmul(o_ps[tj], wp[cc], yfc[:, cc, 512 * tj:512 * (tj + 1)],
                                         start=(cc == 0), stop=(cc == 7))
                nc.gpsimd.collective_compute(
                    "AllGather", mybir.AluOpType.bypass,
                    ins=[y_loc[1][:]], outs=[y_full[1][:]],
                    replica_groups=[list(range(N_CORES))],
                )
                for cc in range(8):
                    nc.gpsimd.dma_start(out=yfc[:, cc, HT:T], in_=yfv[1][:, cc, :])
                for cc in range(8):
                    for tj in (2, 3):
                        nc.tensor.matmul(o_ps[tj], wp[cc], yfc[:, cc, 512 * tj:512 * (tj + 1)],
                                         start=(cc == 0), stop=(cc == 7))
                o_sb = p4o.tile([128, T], F32, tag="osb")
                for tj in range(NT512):
                    nc.scalar.copy(o_sb[:, 512 * tj:512 * (tj + 1)], o_ps[tj])
                nc.gpsimd.dma_start(out=outT[:], in_=o_sb)

    return nc


# ---------------------------------------------------------------------------
# Host-side prep + cached runner
# ---------------------------------------------------------------------------

def _rotary_tables():
    i = np.arange(0, HD, 2, dtype=np.float32)
    inv_freq = (np.float32(1.0) / np.power(np.float32(ROPE_BASE),
                                           i / np.float32(HD))).astype(np.float32)
    t = np.arange(T, dtype=np.float32)
    freqs = t[:, None] * inv_freq[None, :]          # [T, 32]
    cos = np.cos(freqs).astype(np.float32)
    sin = np.sin(freqs).astype(np.float32)
    half = HD // 2
    cosd = np.empty((C, T), np.float32)
    sind = np.empty((C, T), np.float32)
    for h in range(HPC):
        base = HD * h
        cosd[base:base + half] = cos.T
        cosd[base + half:base + HD] = cos.T
        sind[base:base + half] = sin.T
        sind[base + half:base + HD] = -sin.T
    return cosd, sind


def _swap_matrix():
    m = np.zeros((128, 128), np.float32)
    half = HD // 2
    for r in range(128):
        blk, off = divmod(r, HD)
        src = blk * HD + ((off + half) % HD)
        m[src, r] = 1.0
    return m


_CACHE = {}


def _get_runner(repeat=1, phases=4):
    key = f"runner{repeat}_{phases}"
    if key in _CACHE:
        return _CACHE[key]
    _install_patches()
    nc = _build_module(repeat, phases)

    import jax
    import concourse.mybir as mybir
    from jax.sharding import Mesh, PartitionSpec
    from jax.experimental.shard_map import shard_map
    from concourse import bass2jax

    bass2jax.install_neuronx_cc_hook()
    partition_name = nc.partition_id_tensor.name if nc.partition_id_tensor else None
    in_names, out_names, out_avals, zero_outs = [], [], [], []
    for alloc in nc.m.functions[0].allocations:
        if not isinstance(alloc, mybir.MemoryLocationSet):
            continue
        name = alloc.memorylocations[0].name
        if alloc.kind == "ExternalInput":
            if name != partition_name:
                in_names.append(name)
        elif alloc.kind == "ExternalOutput":
            shape = tuple(alloc.tensor_shape)
            dtype = mybir.dt.np(alloc.dtype)
            out_names.append(name)
            out_avals.append(jax.core.ShapedArray(shape, dtype))
            zero_outs.append(np.zeros(shape, dtype))
    all_in_names = in_names + out_names
    if partition_name is not None:
        all_in_names.append(partition_name)
    n_params, n_outs = len(in_names), len(out_avals)

    def _body(*args):
        operands = list(args)
        if partition_name is not None:
            operands.append(bass2jax.partition_id_tensor())
        return tuple(bass2jax._bass_exec_p.bind(
            *operands,
            out_avals=tuple(out_avals),
            in_names=tuple(all_in_names),
            out_names=tuple(out_names),
            lowering_input_output_aliases=(),
            sim_require_finite=True, sim_require_nnan=True, nc=nc,
        ))

    devices = jax.devices()[:N_CORES]
    mesh = Mesh(np.asarray(devices), ("core",))
    fn = jax.jit(
        shard_map(_body, mesh=mesh,
                  in_specs=(PartitionSpec("core"),) * (n_params + n_outs),
                  out_specs=(PartitionSpec("core"),) * n_outs,
                  check_rep=False),
        keep_unused=True,
    )
    state = {
        "fn": fn, "in_names": in_names, "out_names": out_names,
        "out_avals": out_avals, "zero_outs": zero_outs, "nc": nc,
    }
    _CACHE[key] = state
    return state


def _prep_inputs(x, v1, Wq, Wk, Wv, Wproj, lamb, Wgate):
    x = np.asarray(x, np.float32)
    v1 = np.asarray(v1, np.float32)
    lam = np.float32(np.asarray(lamb))
    xT = np.ascontiguousarray(x[0].T)
    cosd, sind = _rotary_tables()
    swapm = _swap_matrix()
    lam1 = np.full((128, 1), np.float32(1.0) - lam, np.float32)
    in_maps = []
    for r in range(N_CORES):
        rows = slice(C * r, C * (r + 1))
        heads = slice(HPC * r, HPC * (r + 1))
        wqkvT = np.ascontiguousarray(
            np.concatenate([np.asarray(Wq)[rows].T, np.asarray(Wk)[rows].T,
                            np.asarray(Wv)[rows].T], axis=1).astype(np.float32))
        in_maps.append({
            "xT": xT,
            "wqkvT": wqkvT,
            "wgT": np.ascontiguousarray(np.asarray(Wgate)[heads].T.astype(np.float32)),
            "wprojT": np.ascontiguousarray(np.asarray(Wproj)[rows].T.astype(np.float32)),
            "v1lam": np.ascontiguousarray((lam * v1[0][:, rows]).astype(np.float32)),
            "lam1": lam1,
            "cosd": cosd,
            "sind": sind,
            "swapm": swapm,
            "identm": np.eye(128, dtype=np.float32),
        })
    return in_maps


def _run(in_maps):
    st = _get_runner()
    concat_in = [
        np.ascontiguousarray(np.concatenate([in_maps[c][n] for c in range(N_CORES)],
                                            axis=0))
        for n in st["in_names"]
    ]
    concat_zeros = [
        np.zeros((N_CORES * z.shape[0], *z.shape[1:]), z.dtype)
        for z in st["zero_outs"]
    ]
    outs = st["fn"](*concat_in, *concat_zeros)
    outs = [np.asarray(o) for o in outs]
    return {n: outs[i].reshape(N_CORES, *st["out_avals"][i].shape)
            for i, n in enumerate(st["out_names"])}


def kernel(x, v1, Wq, Wk, Wv, Wproj, lamb, Wgate):
    in_maps = _prep_inputs(x, v1, Wq, Wk, Wv, Wproj, lamb, Wgate)
    res = _run(in_maps)
    outT = res["outT"]                                     # [cores, C, T]
    y = np.empty((1, T, DIM), np.float32)
    for r in range(N_CORES):
        y[0, :, C * r:C * (r + 1)] = outT[r].T
    return y, np.asarray(v1, np.float32)



# revision 2
# speedup vs baseline: 19.9385x; 19.9385x over previous
"""Trainium2 Bass kernel v2 for nn_CausalSelfAttention_16810501996824.

Head-sharded causal self-attention over 8 NeuronCores, restructured from the
v1 baseline for engine balance:
  - bf16 on every DMA path and matmul operand (f32 PSUM accumulate).
  - Input loads split across the SP + Pool DMA queues to kill the serial
    startup bubble.
  - One activation-table set for the whole kernel: rsqrt = exp(-0.5*ln(x)),
    sigmoid = 1/(1+exp(-x)), so only natural_log_exp_and_others is loaded.
  - V projected directly into [t, head-dim] layout (no 128x128 transposes);
    (1-lambda) folded into Wv, lambda*v1 added from a preloaded slice.
  - Rotary half-swap via DVE stream_shuffle (a 32-partition-group permute)
    instead of a PE swap matmul; RMS-norm computed from the rotated vectors
    (rotation preserves per-head norms).
  - Score matmuls for the core's 2 heads packed into the PE array
    concurrently via tile_position row tiling (K=64 each); both heads'
    scores land in one 2-bank PSUM tile and share one wide Exp.
  - Diagonal attention blocks restricted to their valid column range;
    causal masking via a 0/1 triangle multiply on DVE.
  - P1/P2 window-pipelined; PSUM held to 8 banks (work 2 + scores 4 + y 2).
Phase 4 (AllGather + output-column-sharded c_proj) keeps the v1 structure
with bf16 halving the collective + reload traffic.
"""
import json

import numpy as np

DIM = 1024
N_HEAD = 16
T = 2048
HD = 64
GATE_IN = 12
ROPE_BASE = 10000.0
ATTN_SCALE = 0.1
EPS = 1e-6
N_CORES = 8
HPC = N_HEAD // N_CORES   # 2 heads per core
C = HPC * HD              # 128 channels per core
NT512 = T // 512          # 4 t-windows
NS128 = T // 128          # 16 s-blocks

# stream_shuffle permutes lanes WITHIN each 32-partition block (mask =
# intra-block source lane). Head dims are relabeled so each rotary pair
# (i, i+32) sits 16 apart inside one 32-block:
#   positions [0:16]=orig 0-15(x1)  [16:32]=orig 32-47(x2)
#   positions [32:48]=orig 16-31(x1) [48:64]=orig 48-63(x2)
# so the half-swap is lane+16 mod 32 within every block.
_SWAP_MASK = list(range(16, 32)) + list(range(0, 16))
# position j of a head holds original head-dim _DIM_PERM[j]
_DIM_PERM = (list(range(0, 16)) + list(range(32, 48)) +
             list(range(16, 32)) + list(range(48, 64)))

_WAIT_CAP = {"EventSemaphore": 2}


def _split_multi_waits(bir: dict) -> dict:
    for fn in bir.get("functions", []):
        for blk in fn.get("blocks", []):
            out = []
            changed = False
            for inst in blk.get("instructions", []):
                si = inst.get("sync_info") or {}
                waits = si.get("on_wait") or []
                cap = _WAIT_CAP.get(inst.get("opcode"), 1)
                if len(waits) > cap:
                    changed = True
                    for j, w in enumerate(waits[cap:]):
                        out.append({
                            "debug": inst.get("debug", 0),
                            "engine": inst["engine"],
                            "ins": [], "outs": [],
                            "name": f"{inst['name']}-wsplit{j}",
                            "opcode": "NoOp",
                            "sync_info": {"on_update": [], "on_wait": [w]},
                            "text_hint": "wait_split",
                        })
                    si = dict(si)
                    si["on_wait"] = waits[:cap]
                    inst = dict(inst)
                    inst["sync_info"] = si
                out.append(inst)
            if changed:
                blk["instructions"] = out
    return bir


def _install_patches():
    import concourse.bass as bass
    if getattr(bass.Bass, "_wait_split_patched", False):
        return
    orig = bass.Bass.to_json_bytes

    def patched(self, *a, **k):
        return json.dumps(_split_multi_waits(json.loads(orig(self, *a, **k)))).encode()

    bass.Bass.to_json_bytes = patched
    bass.Bass._wait_split_patched = True


def _build_module(repeat=1, phases=4):
    import concourse.bass as bass
    import concourse.mybir as mybir
    import concourse.tile as tile

    F32 = mybir.dt.float32
    F32R = mybir.dt.float32r
    BF16 = mybir.dt.bfloat16
    AF = mybir.ActivationFunctionType

    nc = bass.Bass()

    xT = nc.declare_dram_parameter("xT", [DIM, T], BF16, isOutput=False)
    wqkvT = nc.declare_dram_parameter("wqkvT", [DIM, 2 * C], BF16, isOutput=False)
    wvT = nc.declare_dram_parameter("wvT", [DIM, C], BF16, isOutput=False)
    wgT = nc.declare_dram_parameter("wgT", [GATE_IN, 33], BF16, isOutput=False)
    selm = nc.declare_dram_parameter("selm", [2, 128], BF16, isOutput=False)
    wprojT = nc.declare_dram_parameter("wprojT", [DIM, C], BF16, isOutput=False)
    v1lam = nc.declare_dram_parameter("v1lam", [T, C], F32, isOutput=False)
    cosd = nc.declare_dram_parameter("cosd", [C, T], BF16, isOutput=False)
    sind = nc.declare_dram_parameter("sind", [C, T], BF16, isOutput=False)
    outT = nc.declare_dram_parameter("outT", [C, T], F32, isOutput=True)

    HT = T // 2
    if phases == 9:
        dbgq = nc.dram_tensor("dbgq", [C, T], BF16)
        dbgk = nc.dram_tensor("dbgk", [C, T], BF16)
        dbgva = nc.dram_tensor("dbgva", [128, NS128, 130], BF16)
    y_loc = [nc.dram_tensor(f"y_loc{i}", [C, HT], BF16) for i in range(2)]
    y_full = [nc.dram_tensor(f"y_full{i}", [N_CORES * C, HT], BF16,
                             addr_space="Shared") for i in range(2)]

    with nc.allow_low_precision(reason="bf16 pipeline"), \
            tile.TileContext(nc) as tc:
      for _rep in range(repeat):
        with tc.tile_pool(name=f"persist{_rep}", bufs=1) as persist, \
             tc.tile_pool(name=f"va{_rep}", bufs=1) as va_pool:
            # ---- persistent SBUF tiles for this rep ----
            qt = persist.tile([128, T], BF16, name="qt")
            kt = persist.tile([128, T], BF16, name="kt")
            y_sb = persist.tile([128, T], BF16, name="y_sb")
            gtmp = persist.tile([33, T], F32R, name="gtmp")
            blh = persist.tile([128, 2], BF16, name="blh")
            sel = persist.tile([2, 128], BF16, name="sel")
            ones_col = persist.tile([65, 64], F32R, name="ones_col")
            eps_sb = persist.tile([128, 1], F32, name="eps_sb")
            tri = persist.tile([128, 128], BF16, name="tri")
            xts = [persist.tile([128, T], BF16, name=f"xt{d}", tag=f"xt{d}")
                   for d in range(8)]
            wts = [persist.tile([128, 2 * C], BF16, name=f"wt{d}", tag=f"wt{d}")
                   for d in range(8)]
            wvs = [persist.tile([128, C], BF16, name=f"wv{d}", tag=f"wv{d}")
                   for d in range(8)]
            wg_sb = persist.tile([GATE_IN, 33], BF16, name="wg_sb")
            cos_sb = persist.tile([128, T], BF16, name="cos_sb")
            sin_sb = persist.tile([128, T], BF16, name="sin_sb")
            v1_sb = persist.tile([128, NS128, C], F32, name="v1_sb")
            wp = [persist.tile([128, C], BF16, name=f"wp{cc}", tag=f"wp{cc}")
                  for cc in range(8)]
            # v_aug per s-block: [h0 dims | ones | h1 dims | ones]
            va = [va_pool.tile([128, 130], BF16, name=f"va{si}", tag=f"va{si}")
                  for si in range(NS128)]

            # ---- input loads, spread across SP (sync) + Pool (gpsimd) ----
            for d in range(4):
                nc.sync.dma_start(out=xts[d], in_=xT[128 * d:128 * (d + 1), :])
                nc.gpsimd.dma_start(out=xts[4 + d],
                                    in_=xT[128 * (4 + d):128 * (5 + d), :])
            for d in range(8):
                eng = nc.sync if d % 2 == 0 else nc.gpsimd
                eng.dma_start(out=wts[d], in_=wqkvT[128 * d:128 * (d + 1), :])
                eng.dma_start(out=wvs[d], in_=wvT[128 * d:128 * (d + 1), :])
            nc.sync.dma_start(out=wg_sb, in_=wgT[:])
            nc.sync.dma_start(out=sel, in_=selm[:])
            nc.sync.dma_start(out=cos_sb, in_=cosd[:])
            nc.sync.dma_start(out=sin_sb, in_=sind[:])
            v1v = v1lam.rearrange("(si p) c -> p si c", p=128)
            nc.gpsimd.dma_start(out=v1_sb, in_=v1v)
            for cc in range(8):
                nc.sync.dma_start(out=wp[cc],
                                  in_=wprojT[128 * cc:128 * (cc + 1), :])

            # ---- constants ----
            nc.vector.memset(blh, 0.0)
            nc.vector.memset(blh[0:64, 0:1], 1.0 / HD)
            nc.vector.memset(blh[64:128, 1:2], 1.0 / HD)
            nc.vector.memset(ones_col.bitcast(F32)[64:65, :], 1.0)
            nc.vector.memset(eps_sb, EPS)
            nc.vector.memset(tri, 1.0)
            nc.gpsimd.affine_select(
                out=tri, in_=tri, compare_op=mybir.AluOpType.is_ge,
                fill=0.0, base=0, channel_multiplier=-1, pattern=[[1, 128]])

            with tc.tile_pool(name="workps", bufs=2, space="PSUM") as work, \
                 tc.tile_pool(name="sps", bufs=2, space="PSUM") as s_pool, \
                 tc.tile_pool(name="yps", bufs=1, space="PSUM") as y_pool, \
                 tc.tile_pool(name="p1t", bufs=2) as p1t, \
                 tc.tile_pool(name="p2t", bufs=4) as p2t, \
                 tc.tile_pool(name="p2s", bufs=3) as p2s:
                def p1(tj):
                    ts = slice(512 * tj, 512 * (tj + 1))
                    # gate: g = 1/(1+exp(-x@WgT))
                    g_ps = work.tile([33, 512], F32, tag="work", name="g_ps")
                    nc.tensor.matmul(g_ps[0:33, :], wg_sb,
                                     xts[0][0:GATE_IN, ts],
                                     start=True, stop=True)
                    ge = p1t.tile([33, 512], F32, tag="ge", name="ge")
                    nc.scalar.activation(ge, g_ps[0:33, :], AF.Exp, scale=-1.0)
                    nc.vector.tensor_scalar_add(ge, ge, 1.0)
                    nc.vector.reciprocal(gtmp[:, ts], ge.bitcast(F32R))

                    # V directly in [t, ch] layout: 4 sub-blocks of 128 t
                    v_ps = work.tile([128, 512], F32, tag="work", name="v_ps")
                    for sub in range(4):
                        si = 4 * tj + sub
                        for d in range(8):
                            nc.tensor.matmul(
                                v_ps[:, 128 * sub:128 * (sub + 1)],
                                xts[d][:, 128 * si:128 * (si + 1)],
                                wvs[d],
                                start=(d == 0), stop=(d == 7))
                    for sub in range(4):
                        si = 4 * tj + sub
                        nc.vector.tensor_add(va[si][:, 0:64],
                                             v_ps[:, 128 * sub:128 * sub + 64],
                                             v1_sb[:, si, 0:64])
                        nc.vector.tensor_add(va[si][:, 65:129],
                                             v_ps[:, 128 * sub + 64:128 * (sub + 1)],
                                             v1_sb[:, si, 64:128])
                        nc.vector.memset(va[si][:, 64:65], 1.0)
                        nc.vector.memset(va[si][:, 129:130], 1.0)

                    # Q, K: PE chains first, then DVE rotary, then norm
                    r_ps = {}
                    for pi, wofs in ((0, 0), (1, C)):
                        r_ps[pi] = work.tile([128, 512], F32, tag="work",
                                             name=f"r_ps{pi}")
                        for d in range(8):
                            nc.tensor.matmul(r_ps[pi],
                                             wts[d][:, wofs:wofs + C],
                                             xts[d][:, ts],
                                             start=(d == 0), stop=(d == 7))
                    sq = {}
                    rotq = {}
                    for pi in (0, 1):
                        raw = p1t.tile([128, 512], BF16, tag=f"raw{pi}",
                                       name=f"raw{pi}")
                        nc.vector.tensor_scalar_mul(raw, r_ps[pi], 1.0)
                        sw = p1t.tile([128, 512], BF16, tag=f"sw{pi}",
                                      name=f"sw{pi}")
                        nc.vector.stream_shuffle(sw, raw, _SWAP_MASK)
                        t1 = p1t.tile([128, 512], BF16, tag=f"t1{pi}",
                                      name=f"t1{pi}")
                        nc.vector.tensor_mul(t1, raw, cos_sb[:, ts])
                        t2 = p1t.tile([128, 512], BF16, tag=f"t2{pi}",
                                      name=f"t2{pi}")
                        nc.vector.tensor_mul(t2, sw, sin_sb[:, ts])
                        rotq[pi] = p1t.tile([128, 512], BF16, tag=f"rot{pi}",
                                            name=f"rot{pi}")
                        nc.vector.tensor_add(rotq[pi], t1, t2)
                        sq[pi] = p1t.tile([128, 512], BF16, tag=f"sq{pi}",
                                          name=f"sq{pi}")
                        nc.vector.tensor_mul(sq[pi], rotq[pi], rotq[pi])
                    ms = {}
                    for pi in (0, 1):
                        ms[pi] = work.tile([2, 512], F32, tag="work",
                                           name=f"ms{pi}")
                        nc.tensor.matmul(ms[pi][0:2, :], blh, sq[pi],
                                         start=True, stop=True)
                    rr = {}
                    for pi in (0, 1):
                        lg = p1t.tile([2, 512], F32, tag=f"lg{pi}",
                                      name=f"lg{pi}")
                        nc.scalar.activation(lg, ms[pi][0:2, :], AF.Ln,
                                             bias=eps_sb[0:2, :])
                        rr[pi] = p1t.tile([2, 512], BF16, tag=f"rr{pi}",
                                          name=f"rr{pi}")
                        nc.scalar.activation(rr[pi], lg, AF.Exp, scale=-0.5)
                    for pi, dst in ((0, qt), (1, kt)):
                        bc_ps = work.tile([128, 512], F32, tag="work",
                                          name=f"bc_ps{pi}")
                        nc.tensor.matmul(bc_ps, sel, rr[pi], start=True,
                                         stop=True)
                        bc_sb = p1t.tile([128, 512], BF16, tag=f"bc{pi}",
                                         name=f"bc{pi}")
                        nc.vector.tensor_scalar_mul(bc_sb, bc_ps, 1.0)
                        nc.vector.tensor_mul(dst[:, ts], rotq[pi], bc_sb)

                def p2(tj):
                    ts = slice(512 * tj, 512 * (tj + 1))
                    nsb = 4 * tj + 4
                    y_ps = [y_pool.tile([65, 512], F32, name=f"y{h}", tag=f"y{h}")
                            for h in range(HPC)]
                    pending = None
                    for si in range(nsb):
                        ss = slice(128 * si, 128 * (si + 1))
                        ko = si - 4 * tj
                        c0 = 128 * ko if ko > 0 else 0
                        s_pair = s_pool.tile([128, 2, 512], F32, tag="s",
                                             name="s_pair")
                        for h in range(HPC):
                            nc.tensor.matmul(
                                s_pair[:, h, c0:512],
                                kt[64 * h:64 * (h + 1), ss],
                                qt[64 * h:64 * (h + 1),
                                   512 * tj + c0:512 * (tj + 1)],
                                start=True, stop=True,
                                tile_position=(64 * h, 0))
                        p_pair = p2t.tile([128, 2, 512], BF16, tag="p",
                                          name="p_pair")
                        nc.scalar.activation(p_pair[:, :, c0:512],
                                             s_pair[:, :, c0:512],
                                             AF.Exp, scale=ATTN_SCALE)
                        if ko >= 0:
                            for h in range(HPC):
                                nc.vector.tensor_mul(p_pair[:, h, c0:c0 + 128],
                                                     p_pair[:, h, c0:c0 + 128],
                                                     tri)
                        if pending is not None:
                            psi, pc0, pp = pending
                            for h in range(HPC):
                                nc.tensor.matmul(
                                    y_ps[h][:, pc0:512],
                                    va[psi][:, 65 * h:65 * h + 65],
                                    pp[:, h, pc0:512],
                                    start=(psi == 0), stop=False)
                        pending = (si, c0, p_pair)
                    psi, pc0, pp = pending
                    for h in range(HPC):
                        nc.tensor.matmul(y_ps[h][:, pc0:512],
                                         va[psi][:, 65 * h:65 * h + 65],
                                         pp[:, h, pc0:512],
                                         start=(psi == 0), stop=True)
                    for h in range(HPC):
                        g64 = p2s.tile([65, 512], F32R, tag="g64", name="g64")
                        nc.sync.dma_start(out=g64[64:65, :],
                                          in_=gtmp[32 * h:32 * h + 1, ts])
                        u = p2s.tile([65, 512], F32R, tag="u", name="u")
                        nc.vector.reciprocal(u[64:65, :],
                                             y_ps[h][64:65, :].bitcast(F32R))
                        cs_row = p2s.tile([65, 512], F32R, tag="cs", name="cs_row")
                        nc.vector.tensor_mul(cs_row[64:65, :], u[64:65, :],
                                             g64[64:65, :])
                        bc2 = work.tile([64, 512], F32, tag="work", name="bc2")
                        nc.tensor.matmul(bc2[0:64, :], ones_col[64:65, :],
                                         cs_row[64:65, :],
                                         start=True, stop=True)
                        cs_sb = p2s.tile([64, 512], F32, tag="csb", name="cs_sb")
                        nc.vector.tensor_scalar_mul(cs_sb, bc2[0:64, :], 1.0)
                        nc.vector.tensor_mul(y_sb[64 * h:64 * (h + 1), ts],
                                             y_ps[h][0:64, :], cs_sb)
                    nc.scalar.dma_start(
                        out=y_loc[tj // 2][:, 512 * (tj % 2):512 * (tj % 2) + 512],
                        in_=y_sb[:, ts])
                    if tj == 1 and phases >= 3:
                        nc.gpsimd.collective_compute(
                            "AllGather", mybir.AluOpType.bypass,
                            ins=[y_loc[0][:]], outs=[y_full[0][:]],
                            replica_groups=[list(range(N_CORES))],
                        )

                # software pipeline: project window tj+1 while attending tj
                p1(0)
                for tj in range(NT512):
                    if tj + 1 < NT512:
                        p1(tj + 1)
                    p2(tj)

            if phases == 9:
                nc.sync.dma_start(out=dbgq[:], in_=qt)
                nc.sync.dma_start(out=dbgk[:], in_=kt)
                for si in range(NS128):
                    nc.sync.dma_start(out=dbgva[:, si, :], in_=va[si])
            if phases < 3:
                nc.sync.dma_start(out=outT[:, 0:HT],
                                  in_=y_sb.bitcast(F32)[:, 0:HT])
                continue

            # ============ P4: output projection ============
            with tc.tile_pool(name="p4", bufs=1) as p4, \
                 tc.tile_pool(name="ops", bufs=1, space="PSUM") as ops_pool:
                o_ps = [ops_pool.tile([128, 512], F32, name=f"o{tj}", tag=f"o{tj}")
                        for tj in range(NT512)]
                yfc = p4.tile([128, 8, T], BF16, tag="yfc", name="yfc")
                yfv = [y_full[i].rearrange("(cc p) t -> p cc t", p=128)
                       for i in range(2)]
                for cc in range(8):
                    nc.gpsimd.dma_start(out=yfc[:, cc, 0:HT], in_=yfv[0][:, cc, :])
                for cc in range(8):
                    for tj in (0, 1):
                        nc.tensor.matmul(o_ps[tj], wp[cc],
                                         yfc[:, cc, 512 * tj:512 * (tj + 1)],
                                         start=(cc == 0), stop=(cc == 7))
                nc.gpsimd.collective_compute(
                    "AllGather", mybir.AluOpType.bypass,
                    ins=[y_loc[1][:]], outs=[y_full[1][:]],
                    replica_groups=[list(range(N_CORES))],
                )
                for cc in range(8):
                    nc.gpsimd.dma_start(out=yfc[:, cc, HT:T], in_=yfv[1][:, cc, :])
                for cc in range(8):
                    for tj in (2, 3):
                        nc.tensor.matmul(o_ps[tj], wp[cc],
                                         yfc[:, cc, 512 * tj:512 * (tj + 1)],
                                         start=(cc == 0), stop=(cc == 7))
                o_sb = p4.tile([128, T], F32, tag="osb", name="o_sb")
                for tj in range(NT512):
                    nc.vector.tensor_scalar_mul(
                        o_sb[:, 512 * tj:512 * (tj + 1)], o_ps[tj], 1.0)
                nc.gpsimd.dma_start(out=outT[:], in_=o_sb)

    return nc


# ---------------------------------------------------------------------------
# Host-side prep + cached runner
# ---------------------------------------------------------------------------

def _rotary_tables():
    i = np.arange(0, HD, 2, dtype=np.float32)
    inv_freq = (np.float32(1.0) / np.power(np.float32(ROPE_BASE),
                                           i / np.float32(HD))).astype(np.float32)
    t = np.arange(T, dtype=np.float32)
    freqs = t[:, None] * inv_freq[None, :]          # [T, 32]
    cos = np.cos(freqs).astype(np.float32).T        # [32, T]
    sin = np.sin(freqs).astype(np.float32).T
    cosd = np.empty((C, T), np.float32)
    sind = np.empty((C, T), np.float32)
    for h in range(HPC):
        base = HD * h
        for j, o in enumerate(_DIM_PERM):
            fi = o if o < 32 else o - 32
            cosd[base + j] = cos[fi]
            sind[base + j] = sin[fi] if o < 32 else -sin[fi]
    return cosd, sind


_CACHE = {}


def _get_runner(repeat=1, phases=4):
    key = f"runner{repeat}_{phases}"
    if key in _CACHE:
        return _CACHE[key]
    _install_patches()
    nc = _build_module(repeat, phases)

    import jax
    import concourse.mybir as mybir
    from jax.sharding import Mesh, PartitionSpec
    from jax.experimental.shard_map import shard_map
    from concourse import bass2jax

    bass2jax.install_neuronx_cc_hook()
    partition_name = nc.partition_id_tensor.name if nc.partition_id_tensor else None
    in_names, out_names, out_avals, zero_outs = [], [], [], []
    for alloc in nc.m.functions[0].allocations:
        if not isinstance(alloc, mybir.MemoryLocationSet):
            continue
        name = alloc.memorylocations[0].name
        if alloc.kind == "ExternalInput":
            if name != partition_name:
                in_names.append(name)
        elif alloc.kind == "ExternalOutput":
            shape = tuple(alloc.tensor_shape)
            dtype = mybir.dt.np(alloc.dtype)
            out_names.append(name)
            out_avals.append(jax.core.ShapedArray(shape, dtype))
            zero_outs.append(np.zeros(shape, dtype))
    all_in_names = in_names + out_names
    if partition_name is not None:
        all_in_names.append(partition_name)
    n_params, n_outs = len(in_names), len(out_avals)

    def _body(*args):
        operands = list(args)
        if partition_name is not None:
            operands.append(bass2jax.partition_id_tensor())
        return tuple(bass2jax._bass_exec_p.bind(
            *operands,
            out_avals=tuple(out_avals),
            in_names=tuple(all_in_names),
            out_names=tuple(out_names),
            lowering_input_output_aliases=(),
            sim_require_finite=True, sim_require_nnan=True, nc=nc,
        ))

    devices = jax.devices()[:N_CORES]
    mesh = Mesh(np.asarray(devices), ("core",))
    fn = jax.jit(
        shard_map(_body, mesh=mesh,
                  in_specs=(PartitionSpec("core"),) * (n_params + n_outs),
                  out_specs=(PartitionSpec("core"),) * n_outs,
                  check_rep=False),
        keep_unused=True,
    )
    state = {
        "fn": fn, "in_names": in_names, "out_names": out_names,
        "out_avals": out_avals, "zero_outs": zero_outs, "nc": nc,
    }
    _CACHE[key] = state
    return state


def _bf16(a):
    import concourse.mybir as mybir
    return np.ascontiguousarray(np.asarray(a).astype(mybir.dt.np(mybir.dt.bfloat16)))


def _prep_inputs(x, v1, Wq, Wk, Wv, Wproj, lamb, Wgate):
    x = np.asarray(x, np.float32)
    v1 = np.asarray(v1, np.float32)
    lam = np.float32(np.asarray(lamb))
    xT = np.ascontiguousarray(x[0].T)
    cosd, sind = _rotary_tables()
    in_maps = []
    for r in range(N_CORES):
        rows = slice(C * r, C * (r + 1))
        heads = slice(HPC * r, HPC * (r + 1))
        perm = np.concatenate([np.asarray(_DIM_PERM) + HD * h for h in range(HPC)])
        wq_p = np.asarray(Wq)[rows][perm]
        wk_p = np.asarray(Wk)[rows][perm]
        wqkvT = np.concatenate([wq_p.T, wk_p.T], axis=1).astype(np.float32)
        wvT = ((np.float32(1.0) - lam) * np.asarray(Wv)[rows].T).astype(np.float32)
        wg_pad = np.zeros((GATE_IN, 33), np.float32)
        wg_pad[:, 0] = np.asarray(Wgate)[heads.start]
        wg_pad[:, 32] = np.asarray(Wgate)[heads.start + 1]
        selm = np.zeros((2, 128), np.float32)
        selm[0, 0:64] = 1.0
        selm[1, 64:128] = 1.0
        in_maps.append({
            "xT": _bf16(xT),
            "wqkvT": _bf16(wqkvT),
            "wvT": _bf16(wvT),
            "wgT": _bf16(wg_pad),
            "selm": _bf16(selm),
            "wprojT": _bf16(np.asarray(Wproj)[rows].T),
            "v1lam": np.ascontiguousarray((lam * v1[0][:, rows]).astype(np.float32)),
            "cosd": _bf16(cosd),
            "sind": _bf16(sind),
        })
    return in_maps


def _run(in_maps):
    st = _get_runner()
    concat_in = [
        np.ascontiguousarray(np.concatenate([in_maps[c][n] for c in range(N_CORES)],
                                            axis=0))
        for n in st["in_names"]
    ]
    concat_zeros = [
        np.zeros((N_CORES * z.shape[0], *z.shape[1:]), z.dtype)
        for z in st["zero_outs"]
    ]
    outs = st["fn"](*concat_in, *concat_zeros)
    outs = [np.asarray(o) for o in outs]
    return {n: outs[i].reshape(N_CORES, *st["out_avals"][i].shape)
            for i, n in enumerate(st["out_names"])}


def kernel(x, v1, Wq, Wk, Wv, Wproj, lamb, Wgate):
    in_maps = _prep_inputs(x, v1, Wq, Wk, Wv, Wproj, lamb, Wgate)
    res = _run(in_maps)
    outT = res["outT"]                                     # [cores, C, T]
    y = np.empty((1, T, DIM), np.float32)
    for r in range(N_CORES):
        y[0, :, C * r:C * (r + 1)] = outT[r].T
    return y, np.asarray(v1, np.float32)
